# revision 1
# baseline (speedup 1.0000x reference)
"""AttentionWithMemory kernel for 8 Trainium2 NeuronCores (Bass/Tile).

Distributed-KNN sharding (classic): the memory bank is sharded 8 x 2048 rows
across cores; each core computes raw similarities for ALL 4096 queries
against its shard (fp16 matmul, fp32 PSUM), takes a local per-query top-8
(DVE max8/max_index), AllGathers packed (val, idx) candidates, and merges the
64 candidates to the exact global top-8 for its own 512 query rows.
Attention runs data-parallel over query rows: scores^T per head via
K^T/Q^T feature-major layouts, unshifted exp (scores are bounded), context
and softmax denominator accumulated in one matmul via a per-head ones column
appended to V, retrieved memory rows gathered by indirect DMA and folded into
the same softmax normalization.

Includes two local workarounds for this container's toolchain (see
_patched_drain_and_barrier / split_sync_waits): walrus here rejects any
instruction carrying more than one semaphore wait.
"""

import numpy as np

import concourse.bass as bass
import concourse.mybir as mybir
from concourse.masks import make_identity
from concourse.tile import TileContext
from concourse.vector_clock import ScopedClock





def _patched_drain_and_barrier(self, tick_clock, wait_clock):
    nc = self.nc
    probe = nc.sync.nop(nofuse=True, hint="tail_drain_waits")
    wait_clock.add_sem_waits(
        probe.ins, ScopedClock({None: tick_clock.global_clock})
    )
    si = probe.ins.sync_info
    if si is not None and len(si.on_wait) > 1:
        waits = list(si.on_wait)
        id2h = {h.num: h for h in self.sems.allocated().values()}
        keep = []
        for w in waits:
            h = id2h.get(w.id)
            if h is not None and w.wait_mode == "sem-ge-imm" and not keep:
                # leave at most one wait on the probe nop, move the rest to
                # standalone waits
                keep.append(w)
            elif h is not None and w.wait_mode == "sem-ge-imm":
                nc.sync.wait_ge(h, w.wait_value)
            else:
                keep.append(w)
        si.on_wait = keep
    nc.sync.drain()

    nc.all_engine_barrier()
    assert self.sems is not None
    popped = nc._tile_sem_poison_stack.pop()
    assert popped is self._sem_poison
    nc.clear_and_free_semaphores(list(self.sems.allocated().values()))
    nc.all_engine_barrier()


def _install_tile_patch():
    TileContext._drain_and_barrier = _patched_drain_and_barrier


_wsplit_counter = [0]


def split_sync_waits(nc, max_waits=1):
    """This container's walrus rejects instructions carrying more than one
    sem-wait ("Too many sync wait commands").  Hoist excess waits onto
    same-engine NoOp instructions inserted directly before the instruction.
    Call after TileContext exit, before running/serializing the module.
    """
    import bass_rust

    fn = nc.m.functions[0]
    for bb in fn.blocks:
        out = []
        changed = False
        for inst in bb.instructions:
            si = inst.sync_info
            if si is not None and len(si.on_wait) > max_waits:
                waits = list(si.on_wait)
                for w in waits[:-max_waits]:
                    _wsplit_counter[0] += 1
                    nop = bass_rust.InstNoOp(name=f"I-wsplit-{_wsplit_counter[0]}")
                    nop.engine = inst.engine
                    nop.sync_info = bass_rust.SyncInfo(on_wait=[w], on_update=[])
                    out.append(nop)
                si.on_wait = waits[-max_waits:]
                changed = True
            out.append(inst)
        if changed:
            bb.instructions = out


F32 = mybir.dt.float32
F16 = mybir.dt.bfloat16  # TEMP bisect
U32 = mybir.dt.uint32
AF = mybir.ActivationFunctionType
OP = mybir.AluOpType
AX = mybir.AxisListType

N_CORES = 8
B, S, E, M = 2, 2048, 1024, 16384
H, HD = 16, 64
NQ = B * S  # 4096
NOWN = NQ // N_CORES  # 512
MSH = M // N_CORES  # 2048
K = 8
EC = E // 128  # 8
SC = S // 128  # 16
QC = NOWN // 128  # 4
AQC = NQ // 128  # 32
SCALE = 1.0 / float(np.sqrt(np.float32(HD)))
HD1 = HD + 1


def build(parts=("cc", "A", "C", "D", "E")):
    nc = bass.Bass("TRN2", target_bir_lowering=False, debug=False, num_devices=N_CORES)

    xT = nc.dram_tensor("xT", [128, EC, NQ], F16, kind="ExternalInput")
    xbT = nc.dram_tensor("xbT", [128, EC, S], F16, kind="ExternalInput")
    xoT = nc.dram_tensor("xoT", [128, EC, NOWN], F16, kind="ExternalInput")
    x_own = nc.dram_tensor("x_own", [NOWN, E], F16, kind="ExternalInput")
    mkT = nc.dram_tensor("mkT", [128, EC, MSH], F16, kind="ExternalInput")
    mv = nc.dram_tensor("mv", [M, E], F16, kind="ExternalInput")
    wqT = nc.dram_tensor("wqT", [128, EC, E], F16, kind="ExternalInput")
    wkT = nc.dram_tensor("wkT", [128, EC, E], F16, kind="ExternalInput")
    wvT = nc.dram_tensor("wvT", [128, EC, E], F16, kind="ExternalInput")
    woT = nc.dram_tensor("woT", [128, EC, E], F16, kind="ExternalInput")
    bq_c = nc.dram_tensor("bq_c", [128, EC], F32, kind="ExternalInput")
    bk_c = nc.dram_tensor("bk_c", [128, EC], F32, kind="ExternalInput")
    bv_r = nc.dram_tensor("bv_r", [1, E], F32, kind="ExternalInput")
    bo_r = nc.dram_tensor("bo_r", [1, E], F32, kind="ExternalInput")
    ownidx = nc.dram_tensor("ownidx", [128, QC, N_CORES], U32, kind="ExternalInput")
    offs = nc.dram_tensor("offs", [1, N_CORES * K], F32, kind="ExternalInput")

    out = nc.dram_tensor("out", [NOWN, E], F32, kind="ExternalOutput")

    with TileContext(nc) as tc:
        with (
            tc.tile_pool(name="pp", bufs=1) as pp,
            tc.tile_pool(name="big", bufs=1) as bigp,
            tc.tile_pool(name="wbuf", bufs=1) as wp,
            tc.tile_pool(name="lp1", bufs=1) as lp1,
            tc.tile_pool(name="lp2", bufs=2) as lp2,
            tc.tile_pool(name="simsp", bufs=2) as simsp,
            tc.tile_pool(name="ps", bufs=2, space="PSUM") as ps,
            tc.tile_pool(name="ps2", bufs=2, space="PSUM") as ps2,
            tc.tile_pool(name="dram", bufs=1, space="DRAM") as dr,
        ):
            # ================= loads =================
            mkT_sb = bigp.tile([128, EC, MSH], F16, tag="bigC")
            nc.sync.dma_start(mkT_sb[:], mkT[:])

            ones128 = pp.tile([128, 1], F32)
            nc.vector.memset(ones128[:], 1.0)
            ones1 = pp.tile([1, 128], F32)
            nc.vector.memset(ones1[:], 1.0)

            # ========== kn inverse norms; scale mkT columns in place ==========
            n2_ps = [
                ps.tile([1, 512], F32, tag=f"m{mc % 2}", name=f"n2_{mc}")
                for mc in range(4)
            ]
            for kc in range(EC):
                sq = simsp.tile([128, MSH], F32, tag="sims_big")
                nc.vector.tensor_tensor(
                    sq[:], mkT_sb[:, kc, :], mkT_sb[:, kc, :], OP.mult
                )
                for mc in range(4):
                    nc.tensor.matmul(
                        n2_ps[mc][:],
                        lhsT=ones128[:],
                        rhs=sq[:, mc * 512 : (mc + 1) * 512],
                        start=(kc == 0),
                        stop=(kc == EC - 1),
                    )
            inv_kn = pp.tile([1, MSH], F32)
            for mc in range(4):
                nc.scalar.activation(
                    inv_kn[:, mc * 512 : (mc + 1) * 512], n2_ps[mc][:], AF.Sqrt
                )
            nc.vector.reciprocal(inv_kn[:], inv_kn[:])
            inv_rep = pp.tile([128, MSH], F16)
            for mc in range(4):
                r_ps = ps2.tile([128, 512], F32, tag="p2")
                nc.tensor.matmul(
                    r_ps[:],
                    lhsT=ones1[:],
                    rhs=inv_kn[:, mc * 512 : (mc + 1) * 512],
                    start=True,
                    stop=True,
                )
                nc.vector.tensor_copy(inv_rep[:, mc * 512 : (mc + 1) * 512], r_ps[:])
            # ========== qn inverse norms (own queries) ==========
            inv_qn = pp.tile([128, QC], F32)
            for qc in range(QC):
                xo = lp1.tile([128, E], F16, tag="xo")
                nc.sync.dma_start(xo[:], x_own[qc * 128 : (qc + 1) * 128, :])
                trash = lp1.tile([128, E], F16, tag="xo_trash")
                nrm2 = lp1.tile([128, 1], F32, tag="nrm2")
                nc.scalar.activation(trash[:], xo[:], AF.Square, accum_out=nrm2[:])
                nc.scalar.activation(inv_qn[:, qc : qc + 1], nrm2[:], AF.Sqrt)
            nc.vector.reciprocal(inv_qn[:], inv_qn[:])

            # ========== phase B: sims + local top-8 + candidate pack ==========
            cand_dr = dr.tile([NQ, 2 * K], F32)
            HQ = NQ // 2
            for half in range(2):
                xh = bigp.tile([128, EC, HQ], F16, tag="bigA")
                nc.sync.dma_start(xh[:], xT[:, :, half * HQ : (half + 1) * HQ])
                for qh in range(HQ // 128):
                    qc = half * (HQ // 128) + qh
                    sims = simsp.tile([128, MSH], F32, tag="sims_big")
                    for mh in range(2):
                        sp0 = ps.tile([128, 512], F32, tag="m0", name=f"sp0_{qc}_{mh}")
                        sp1 = ps.tile([128, 512], F32, tag="m1", name=f"sp1_{qc}_{mh}")
                        for kc in range(EC):
                            for mc2, spt in ((0, sp0), (1, sp1)):
                                mc = mh * 2 + mc2
                                nc.tensor.matmul(
                                    spt[:],
                                    lhsT=xh[:, kc, qh * 128 : (qh + 1) * 128],
                                    rhs=mkT_sb[:, kc, mc * 512 : (mc + 1) * 512],
                                    start=(kc == 0),
                                    stop=(kc == EC - 1),
                                )
                        for mc2, spt in ((0, sp0), (1, sp1)):
                            mc = mh * 2 + mc2
                            nc.vector.tensor_tensor(
                                sims[:, mc * 512 : (mc + 1) * 512],
                                spt[:],
                                inv_rep[:, mc * 512 : (mc + 1) * 512],
                                OP.mult,
                            )
                    lv = simsp.tile([128, K], F32, tag="lv")
                    li = simsp.tile([128, K], U32, tag="li")
                    nc.vector.max(lv[:], sims[:])
                    nc.vector.max_index(li[:], lv[:], sims[:])
                    cand = simsp.tile([128, 2 * K], F32, tag="cand")
                    nc.vector.tensor_copy(cand[:, :K], lv[:])
                    nc.vector.tensor_copy(cand[:, K:], li[:])
                    nc.sync.dma_start(cand_dr[qc * 128 : (qc + 1) * 128, :], cand[:])

            # ================= AllGather =================
            ag_sh = nc.dram_tensor(
                "ag_sh", [N_CORES * NQ, 2 * K], F32, addr_space="Shared"
            )
            ag_dr = ag_sh
            if "cc" in parts:
                nc.gpsimd.collective_compute(
                    "AllGather",
                    OP.bypass,
                    replica_groups=[list(range(N_CORES))],
                    ins=[cand_dr[:].opt()],
                    outs=[ag_dr.ap().opt()],
                )

            # ========== phase A: projections ==========
            do_A = "A" in parts
            if not do_A:
                z_sb = pp.tile([128, E], F32)
                nc.vector.memset(z_sb[:], 0.0)
                for qc in range(QC):
                    nc.sync.dma_start(out[qc * 128 : (qc + 1) * 128, :], z_sb[:])
                return _finish(tc)
            xbT_sb = bigp.tile([128, EC, S], F16, tag="bigB")
            nc.sync.dma_start(xbT_sb[:], xbT[:])
            xoT_sb = bigp.tile([128, EC, NOWN], F16, tag="bigD")
            nc.sync.dma_start(xoT_sb[:], xoT[:])
            bq_sb = pp.tile([128, EC], F32)
            nc.sync.dma_start(bq_sb[:], bq_c[:])
            bk_sb = pp.tile([128, EC], F32)
            nc.sync.dma_start(bk_sb[:], bk_c[:])
            bv_sb = pp.tile([1, E], F32)
            nc.sync.dma_start(bv_sb[:], bv_r[:])
            bo_sb = pp.tile([1, E], F32)
            nc.sync.dma_start(bo_sb[:], bo_r[:])
            bv_rep = pp.tile([128, E], F32)
            for eo in range(2):
                b_ps = ps2.tile([128, 512], F32, tag="p2")
                nc.tensor.matmul(
                    b_ps[:],
                    lhsT=ones1[:],
                    rhs=bv_sb[:, eo * 512 : (eo + 1) * 512],
                    start=True,
                    stop=True,
                )
                nc.vector.tensor_copy(bv_rep[:, eo * 512 : (eo + 1) * 512], b_ps[:])
            # Q^T
            QT = pp.tile([128, EC, NOWN], F16)
            for wh in range(2):
                wq_sb = wp.tile([128, EC, E // 2], F16, tag="w", name=f"wq_{wh}")
                nc.sync.dma_start(wq_sb[:], wqT[:, :, wh * 512 : (wh + 1) * 512])
                for eo4 in range(4):
                    eo = wh * 4 + eo4
                    q_ps = ps2.tile([128, NOWN], F32, tag="p2")
                    for kc in range(EC):
                        nc.tensor.matmul(
                            q_ps[:],
                            lhsT=wq_sb[:, kc, eo4 * 128 : (eo4 + 1) * 128],
                            rhs=xoT_sb[:, kc, :],
                            start=(kc == 0),
                            stop=(kc == EC - 1),
                        )
                    nc.scalar.activation(
                        QT[:, eo, :], q_ps[:], AF.Identity, bias=bq_sb[:, eo : eo + 1]
                    )

            # V natural, with per-head trailing ones column
            V_ext = bigp.tile([128, SC, H * HD1], F16, tag="bigA")
            for sc in range(SC):
                nc.vector.memset(
                    V_ext[:, sc, :].rearrange("p (h e) -> p h e", h=H)[:, :, HD], 1.0
                )
            for eo in range(2):
                wv_sb = wp.tile([128, EC, E // 2], F16, tag="w", name=f"wv_{eo}")
                nc.sync.dma_start(wv_sb[:], wvT[:, :, eo * 512 : (eo + 1) * 512])
                for sc in range(SC):
                    v_ps = ps2.tile([128, 512], F32, tag="p2")
                    for kc in range(EC):
                        nc.tensor.matmul(
                            v_ps[:],
                            lhsT=xbT_sb[:, kc, sc * 128 : (sc + 1) * 128],
                            rhs=wv_sb[:, kc, :],
                            start=(kc == 0),
                            stop=(kc == EC - 1),
                        )
                    dst = V_ext[:, sc, eo * 8 * HD1 : (eo * 8 + 8) * HD1]
                    nc.vector.tensor_tensor(
                        dst.rearrange("p (h e) -> p h e", h=8)[:, :, :HD],
                        v_ps[:].rearrange("p (h e) -> p h e", h=8),
                        bv_rep[:, eo * 512 : (eo + 1) * 512].rearrange(
                            "p (h e) -> p h e", h=8
                        ),
                        OP.add,
                    )

            # K^T (into mkT's slot; waits for sims to finish reading mkT)
            KT = bigp.tile([128, EC, S], F16, tag="bigC")
            for wh in range(2):
                wk_sb = wp.tile([128, EC, E // 2], F16, tag="w", name=f"wk_{wh}")
                nc.sync.dma_start(wk_sb[:], wkT[:, :, wh * 512 : (wh + 1) * 512])
                for eo4 in range(4):
                    eo = wh * 4 + eo4
                    for sc4 in range(S // 512):
                        k_ps = ps2.tile([128, 512], F32, tag="p2")
                        for kc in range(EC):
                            nc.tensor.matmul(
                                k_ps[:],
                                lhsT=wk_sb[:, kc, eo4 * 128 : (eo4 + 1) * 128],
                                rhs=xbT_sb[:, kc, sc4 * 512 : (sc4 + 1) * 512],
                                start=(kc == 0),
                                stop=(kc == EC - 1),
                            )
                        nc.scalar.activation(
                            KT[:, eo, sc4 * 512 : (sc4 + 1) * 512],
                            k_ps[:],
                            AF.Identity,
                            bias=bk_sb[:, eo : eo + 1],
                        )

            # ========== phase C: merge + memory path ==========
            ctx_acc = pp.tile([128, QC, H * HD1], F16)
            ctx_mem = pp.tile([128, QC, E], F16)
            do_C = "C" in parts
            ownidx_sb = pp.tile([128, QC, N_CORES], U32)
            nc.sync.dma_start(ownidx_sb[:], ownidx[:])
            offs_sb = pp.tile([1, N_CORES * K], F32)
            nc.sync.dma_start(offs_sb[:], offs[:])
            o_ps = ps2.tile([128, N_CORES * K], F32, tag="p2")
            nc.tensor.matmul(
                o_ps[:], lhsT=ones1[:], rhs=offs_sb[:], start=True, stop=True
            )
            offs_rep = pp.tile([128, N_CORES * K], F32)
            nc.vector.tensor_copy(offs_rep[:], o_ps[:])

            l_mem = pp.tile([128, QC], F32)
            exp_mem = pp.tile([128, QC, K], F32)
            gidx = pp.tile([128, QC, K], U32)
            if not do_C:
                nc.vector.memset(l_mem[:], 0.0)
                nc.vector.memset(ctx_mem[:], 0.0)
            for qc in range(QC if do_C else 0):
                own = lp2.tile([128, N_CORES, 2 * K], F32, tag="own")
                for r in range(N_CORES):
                    nc.gpsimd.indirect_dma_start(
                        out=own[:, r, :],
                        out_offset=None,
                        in_=ag_dr.ap(),
                        in_offset=bass.IndirectOffsetOnAxis(
                            ap=ownidx_sb[:, qc, r : r + 1], axis=0
                        ),
                    )
                mvv = lp2.tile([128, N_CORES * K], F32, tag="mvv")
                mii = lp2.tile([128, N_CORES * K], F32, tag="mii")
                nc.vector.tensor_copy(
                    mvv[:].rearrange("p (r k) -> p r k", r=N_CORES), own[:, :, :K]
                )
                nc.vector.tensor_tensor(
                    mii[:].rearrange("p (r k) -> p r k", r=N_CORES),
                    own[:, :, K:],
                    offs_rep[:].rearrange("p (r k) -> p r k", r=N_CORES),
                    OP.add,
                )
                gv = lp2.tile([128, K], F32, tag="gv")
                nc.vector.max(gv[:], mvv[:])
                gi_f = lp2.tile([128, K], F32, tag="gi_f")
                for k in range(K):
                    eq = lp2.tile([128, N_CORES * K], F32, tag="eq")
                    nc.vector.tensor_tensor(
                        eq[:],
                        mvv[:],
                        gv[:, k : k + 1].to_broadcast([128, N_CORES * K]),
                        OP.is_equal,
                    )
                    nc.vector.tensor_tensor(eq[:], eq[:], mii[:], OP.mult)
                    nc.vector.reduce_max(gi_f[:, k : k + 1], eq[:], axis=AX.X)
                nc.vector.tensor_copy(gidx[:, qc, :], gi_f[:])
                tv = lp2.tile([128, K], F32, tag="tv")
                nc.vector.tensor_tensor(
                    tv[:], gv[:], inv_qn[:, qc : qc + 1].to_broadcast([128, K]), OP.mult
                )
                nc.scalar.activation(exp_mem[:, qc, :], tv[:], AF.Exp)
                nc.vector.reduce_sum(
                    l_mem[:, qc : qc + 1], exp_mem[:, qc, :], axis=AX.X
                )
                cm = ctx_mem[:, qc, :]
                nc.vector.memset(cm, 0.0)
                for k in range(K):
                    ret = lp2.tile([128, E], F16, tag="ret")
                    nc.gpsimd.indirect_dma_start(
                        out=ret[:],
                        out_offset=None,
                        in_=mv[:],
                        in_offset=bass.IndirectOffsetOnAxis(
                            ap=gidx[:, qc, k : k + 1], axis=0
                        ),
                    )
                    ret_s = lp2.tile([128, E], F16, tag="ret_s")
                    nc.scalar.activation(
                        ret_s[:], ret[:], AF.Copy, scale=exp_mem[:, qc, k : k + 1]
                    )
                    nc.vector.tensor_tensor(cm, cm, ret_s[:], OP.add)

            # ========== phase D: scores^T, exp, ctx accumulation ==========
            if "D" not in parts:
                z_sb = pp.tile([128, E], F32)
                nc.vector.memset(z_sb[:], 0.0)
                for qc in range(QC):
                    nc.sync.dma_start(out[qc * 128 : (qc + 1) * 128, :], z_sb[:])
                return _finish(tc)
            for h in range(H):
                po = (h % 2) * 64
                eo = h // 2
                c_ps = [
                    ps.tile([128, HD1], F32, tag=f"m{qc % 2}", name=f"cps_{h}_{qc}")
                    for qc in range(QC)
                ]
                for sc in range(SC):
                    s_ps = ps2.tile([128, NOWN], F32, tag="p2")
                    nc.tensor.matmul(
                        s_ps[:],
                        lhsT=KT[po : po + 64, eo, sc * 128 : (sc + 1) * 128],
                        rhs=QT[po : po + 64, eo, :],
                        start=True,
                        stop=True,
                    )
                    expT = lp2.tile([128, NOWN], F16, tag="expT")
                    nc.scalar.activation(expT[:], s_ps[:], AF.Exp, scale=SCALE)
                    for qc in range(QC):
                        nc.tensor.matmul(
                            c_ps[qc][:],
                            lhsT=expT[:, qc * 128 : (qc + 1) * 128],
                            rhs=V_ext[:, sc, h * HD1 : (h + 1) * HD1],
                            start=(sc == 0),
                            stop=(sc == SC - 1),
                        )
                for qc in range(QC):
                    nc.scalar.activation(
                        ctx_acc[:, qc, h * HD1 : (h + 1) * HD1], c_ps[qc][:], AF.Copy
                    )

            # ========== phase E: combine, normalize, transpose, Wo ==========
            if "E" not in parts:
                z_sb = pp.tile([128, E], F32)
                nc.vector.memset(z_sb[:], 0.0)
                for qc in range(QC):
                    nc.sync.dma_start(out[qc * 128 : (qc + 1) * 128, :], z_sb[:])
                return _finish(tc)
            ident = pp.tile([128, 128], F16)
            make_identity(nc, ident[:])
            bo_rep = pp.tile([128, E], F32)
            for eo in range(2):
                b_ps = ps2.tile([128, 512], F32, tag="p2")
                nc.tensor.matmul(
                    b_ps[:],
                    lhsT=ones1[:],
                    rhs=bo_sb[:, eo * 512 : (eo + 1) * 512],
                    start=True,
                    stop=True,
                )
                nc.vector.tensor_copy(bo_rep[:, eo * 512 : (eo + 1) * 512], b_ps[:])
            ctxT = bigp.tile([128, EC, NOWN], F16, tag="bigD")
            for qc in range(QC):
                acc_h = ctx_acc[:, qc, :].rearrange("p (h e) -> p h e", h=H)
                lt = lp1.tile([128, H], F32, tag="lt")
                nc.vector.tensor_tensor(
                    lt[:],
                    acc_h[:, :, HD],
                    l_mem[:, qc : qc + 1].to_broadcast([128, H]),
                    OP.add,
                )
                rec = lp1.tile([128, H], F32, tag="rec")
                nc.vector.reciprocal(rec[:], lt[:])
                tmp = lp1.tile([128, E], F32, tag="tmp")
                nc.vector.tensor_tensor(
                    tmp[:].rearrange("p (h e) -> p h e", h=H),
                    acc_h[:, :, :HD],
                    ctx_mem[:, qc, :].rearrange("p (h e) -> p h e", h=H),
                    OP.add,
                )
                ctx_n = lp1.tile([128, E], F16, tag="ctx_n")
                for h in range(H):
                    nc.scalar.activation(
                        ctx_n[:, h * HD : (h + 1) * HD],
                        tmp[:, h * HD : (h + 1) * HD],
                        AF.Copy,
                        scale=rec[:, h : h + 1],
                    )
                for eo in range(EC):
                    t_ps = ps2.tile([128, 128], F16, tag="p2")
                    nc.tensor.transpose(
                        t_ps[:], ctx_n[:, eo * 128 : (eo + 1) * 128], ident[:]
                    )
                    nc.vector.tensor_copy(
                        ctxT[:, eo, qc * 128 : (qc + 1) * 128], t_ps[:]
                    )

            for eo in range(2):
                wo_sb = wp.tile([128, EC, E // 2], F16, tag="w", name=f"wo_{eo}")
                nc.sync.dma_start(wo_sb[:], woT[:, :, eo * 512 : (eo + 1) * 512])
                for qc in range(QC):
                    o_sb = lp1.tile([128, 512], F32, tag="o_sb")
                    w_ps = ps2.tile([128, 512], F32, tag="p2")
                    for kc in range(EC):
                        nc.tensor.matmul(
                            w_ps[:],
                            lhsT=ctxT[:, kc, qc * 128 : (qc + 1) * 128],
                            rhs=wo_sb[:, kc, :],
                            start=(kc == 0),
                            stop=(kc == EC - 1),
                        )
                    nc.vector.tensor_tensor(
                        o_sb[:],
                        w_ps[:],
                        bo_rep[:, eo * 512 : (eo + 1) * 512],
                        OP.add,
                    )
                    nc.sync.dma_start(
                        out[qc * 128 : (qc + 1) * 128, eo * 512 : (eo + 1) * 512],
                        o_sb[:],
                    )

    return nc


def _finish(tc):
    return tc.nc


def chunkT(a):
    """[n, 1024] fp32 -> [128, 8, n] (a^T, feature-chunked) fp16."""
    n = a.shape[0]
    return np.ascontiguousarray(
        a.T.reshape(EC, 128, n).transpose(1, 0, 2)
    ).astype(__import__('ml_dtypes').bfloat16)


def prep_inputs(np_inputs):
    """Host-side layout prep. Returns per-core in_maps list."""
    f16 = __import__('ml_dtypes').bfloat16
    hidden = np.asarray(np_inputs["hidden_states"], np.float32)
    mk = np.asarray(np_inputs["memory_keys"], np.float32)
    mvv = np.asarray(np_inputs["memory_values"], np.float32)
    top_k = int(np.asarray(np_inputs["top_k"]))
    assert top_k == K, f"kernel hardcodes top_k=8, got {top_k}"

    x_flat = hidden.reshape(NQ, E)
    xT_all = chunkT(x_flat)
    mv_bf = mvv.astype(f16)
    w_t = {
        n: chunkT(np.asarray(np_inputs[n], np.float32))
        for n in ("Wq", "Wk", "Wv", "Wo")
    }
    bq = np.asarray(np_inputs["bq"], np.float32)
    bk = np.asarray(np_inputs["bk"], np.float32)
    bq_ck = np.ascontiguousarray(bq.reshape(EC, 128).T)
    bk_ck = np.ascontiguousarray(bk.reshape(EC, 128).T)
    bv_r = np.asarray(np_inputs["bv"], np.float32)[None, :]
    bo_r = np.asarray(np_inputs["bo"], np.float32)[None, :]
    offs_arr = np.repeat(np.arange(N_CORES) * MSH, K).astype(np.float32)[None, :]

    in_maps = []
    for c in range(N_CORES):
        b = c // (N_CORES // B)
        own0 = c * NOWN
        ownidx_c = np.zeros((128, QC, N_CORES), np.uint32)
        for qc in range(QC):
            rows = own0 + qc * 128 + np.arange(128)
            for r in range(N_CORES):
                ownidx_c[:, qc, r] = r * NQ + rows
        in_maps.append(
            {
                "xT": xT_all,
                "xbT": np.ascontiguousarray(xT_all[:, :, b * S : (b + 1) * S]),
                "xoT": np.ascontiguousarray(xT_all[:, :, own0 : own0 + NOWN]),
                "x_own": x_flat[own0 : own0 + NOWN].astype(f16),
                "mkT": chunkT(mk[c * MSH : (c + 1) * MSH]),
                "mv": mv_bf,
                "wqT": w_t["Wq"],
                "wkT": w_t["Wk"],
                "wvT": w_t["Wv"],
                "woT": w_t["Wo"],
                "bq_c": bq_ck,
                "bk_c": bk_ck,
                "bv_r": bv_r,
                "bo_r": bo_r,
                "ownidx": ownidx_c,
                "offs": offs_arr,
            }
        )
    return in_maps


def assemble(results):
    """Concat per-core [512, 1024] outputs to [B, S, E]."""
    outs = [np.asarray(r["out"]) for r in results]
    return np.concatenate(outs, axis=0).reshape(B, S, E).astype(np.float32)


_CACHE = {}


def _get_module():
    if "nc" not in _CACHE:
        _install_tile_patch()
        nc = build()
        split_sync_waits(nc)
        _CACHE["nc"] = nc
    return _CACHE["nc"]


def kernel(**inputs):
    from concourse.bass_utils import run_bass_kernel_spmd

    nc = _get_module()
    in_maps = prep_inputs(inputs)
    res = run_bass_kernel_spmd(nc, in_maps, core_ids=list(range(N_CORES)))
    return assemble(res.results)

